# revision 44
# baseline (speedup 1.0000x reference)
"""Multi-head causal attention (B=2, S=2048, D=1024, H=16) on 8 NeuronCores.

Sharding: data-parallel over batch (2) x tensor-parallel over head groups
(4 groups of 4 heads).  Each core computes QKV projections for its head
slice, causal attention for its 4 heads, and a partial output projection;
the host sums the 4 head-group partials per batch and adds the bias.

v2: all matmul operands are bf16 (full PE rate at any tile width, half the
HBM traffic of f32r), PSUM accumulation stays f32, output partials are
fp16.  x streams in per 512-column chunk so chunk-0 projections start as
soon as the first megabyte lands; dummy warm-up matmuls keep the PE
p-state ramp on otherwise-idle time.  Scores are computed transposed
(keys on partitions) so softmax denominators come from a ones-column
appended to V.  Causal structure: sub-diagonal 128x512 score tiles are
skipped, diagonal tiles are computed only on their valid column range,
and the single 128x128 boundary block gets a bf16 mask multiply on DVE
(2x mode).  Softmax normalization reads ctx straight from PSUM.
"""

import numpy as np
import ml_dtypes

import concourse.bacc as bacc
import concourse.mybir as mybir
import concourse.tile as tile
from concourse import bass_utils
from concourse.bass import ds, ts


def ds512(i, lo):
    return ds(i * 512 + lo, 512 - lo)


f32 = mybir.dt.float32
bf16 = mybir.dt.bfloat16
f16 = mybir.dt.float16
AFT = mybir.ActivationFunctionType

B, S, D, H = 2, 2048, 1024, 16
HD = D // H          # 64
NCORES = 8
NG = 4               # head groups (cores per batch)
GH = H // NG         # heads per core = 4
GO = GH * HD         # output channels per core = 256
KT = D // 128        # 8 k-tiles over the model dim
SC = S // 512        # 4 query chunks of 512
SJ = S // 128        # 16 key tiles of 128

_CACHE = {}
MARKS = []  # (instruction-id watermark, label) for trace attribution


def _build():
    MARKS.clear()
    nc = bacc.Bacc(None)

    def mark(label):
        MARKS.append((nc.next_id(), label))
    xT_d = nc.dram_tensor("xT", [D, S], bf16, kind="ExternalInput")
    wq_d = nc.dram_tensor("wqT", [D, GO], bf16, kind="ExternalInput")
    wk_d = nc.dram_tensor("wkT", [D, GO], bf16, kind="ExternalInput")
    wv_d = nc.dram_tensor("wvT", [D, GO], bf16, kind="ExternalInput")
    wo_d = nc.dram_tensor("woT", [GO, D], bf16, kind="ExternalInput")
    mask_d = nc.dram_tensor("mask", [128, 2, 128], bf16, kind="ExternalInput")
    out_d = nc.dram_tensor("out", [S, D], f16, kind="ExternalOutput")

    with tile.TileContext(nc) as tc:
        with tc.tile_pool(name="const", bufs=1) as constp, \
             tc.tile_pool(name="big", bufs=1) as bigp, \
             tc.tile_pool(name="probs", bufs=5) as probsp, \
             tc.tile_pool(name="osb", bufs=8) as osbp, \
             tc.tile_pool(name="mm_ps", bufs=2, space="PSUM") as mmps, \
             tc.tile_pool(name="sc_ps", bufs=2, space="PSUM") as scps, \
             tc.tile_pool(name="ctx_ps", bufs=1, space="PSUM") as ctxps:

            wq_t = constp.tile([128, KT, GO], bf16)
            wk_t = constp.tile([128, KT, GO], bf16)
            wv_t = constp.tile([128, KT, GO], bf16)
            wo_t = constp.tile([128, 2, D], bf16)
            mask_t = constp.tile([128, 2, 128], bf16)
            xT_t = bigp.tile([128, KT, S], bf16)
            xT_src = xT_d.rearrange("(k p) s -> p k s", p=128)

            # Input DMA order follows first-use: wk, x chunk 0 (split in two
            # k-halves for an earlier first matmul), wq, wv, then the
            # remaining x chunks and wo.  Spread over SP HWDGE, ACT HWDGE and
            # Pool SWDGE so descriptor generation pipelines.
            mark("input-dma")
            nc.sync.dma_start(wk_t[:], wk_d.rearrange("(k p) o -> p k o", p=128))
            nc.sync.dma_start(xT_t[:, 0:4, ts(0, 512)], xT_src[:, 0:4, ts(0, 512)])
            nc.sync.dma_start(xT_t[:, 4:8, ts(0, 512)], xT_src[:, 4:8, ts(0, 512)])
            nc.scalar.dma_start(wq_t[:], wq_d.rearrange("(k p) o -> p k o", p=128))
            nc.gpsimd.dma_start(wv_t[:], wv_d.rearrange("(k p) o -> p k o", p=128))
            nc.gpsimd.dma_start(mask_t[:], mask_d[:])
            nc.sync.dma_start(xT_t[:, :, ts(1, 512)], xT_src[:, :, ts(1, 512)])
            nc.scalar.dma_start(xT_t[:, :, ts(2, 512)], xT_src[:, :, ts(2, 512)])
            nc.gpsimd.dma_start(wo_t[:], wo_d.rearrange("(t p) n -> p t n", p=128))
            nc.gpsimd.dma_start(xT_t[:, :, ts(3, 512)], xT_src[:, :, ts(3, 512)])

            # PE p-state warm-up: the cost model runs the PE at 0.65/1.2 GHz
            # until 3us of continuous-ish (gaps < 3us) busy time accrues.
            # Burn the input-DMA wait on dummy matmuls so real work starts
            # at 2.4 GHz.
            mark("warmup")
            warm = constp.tile([128, 512], bf16)
            nc.vector.memset(warm[:], 0.0)
            ones_f = constp.tile([128, SJ * GH], bf16)
            nc.vector.memset(ones_f[:], 1.0)
            for _ in range(9):
                wps = mmps.tile([128, 512], f32, tag="mm", name="mm")
                nc.tensor.matmul(wps[:], warm[:, 0:128], warm[:],
                                 start=True, stop=True)

            QT_t = bigp.tile([128, 2, S], bf16)     # [o, s] head-major
            KTr_t = bigp.tile([128, 2, S], bf16)
            V_t = bigp.tile([128, SJ, GH, HD + 1], bf16)  # V cols + ones
            ctxT_t = bigp.tile([128, 2, S], bf16)

            nc.vector.tensor_copy(
                V_t[:, :, :, HD],
                ones_f[:].rearrange("p (a b) -> p a b", b=GH))

            # --- emission-order-interleaved pipeline ---
            def kq_group(w_t, dst, t, c):
                mark(f"kq t{t} c{c} {'K' if w_t is wk_t else 'Q'}")
                mm = mmps.tile([128, 512], f32, tag="mm", name="mm")
                for k in range(KT):
                    nc.tensor.matmul(
                        mm[:], w_t[:, k, ts(t, 128)], xT_t[:, k, ts(c, 512)],
                        start=(k == 0), stop=(k == KT - 1))
                nc.vector.tensor_copy(dst[:, t, ts(c, 512)], mm[:])

            def v_group(jt):
                mark(f"v jt{jt}")
                mm = mmps.tile([128, 512], f32, tag="mm", name="mm")
                for k in range(KT):
                    nc.tensor.matmul(
                        mm[:, 0:GO], xT_t[:, k, ts(jt, 128)], wv_t[:, k, :],
                        start=(k == 0), stop=(k == KT - 1))
                src = mm[:, 0:GO].rearrange("p (h e) -> p h e", e=HD)
                nc.vector.tensor_copy(V_t[:, jt, :, 0:HD], src)

            def attn_pair(i, hp, on_piece=None, defer_norm=False,
                          early_filler=(), mid_hooks=None):
                # heads h0 = 2*hp, h1 = 2*hp+1 share QT/KT tile t=hp with
                # partition offsets 0 and 64.  With on_piece set, softmax
                # normalization is emitted per 128-column block as soon as
                # that block's ctx accumulation is complete (all j-tiles with
                # lo <= block start have run), and on_piece(p) emits the
                # dependent output-projection work right behind it — keeps
                # the end-of-kernel dependency chain short.
                to = hp
                heads = (2 * hp, 2 * hp + 1)
                cps = ctxps.tile([128, 2, 512], f32, tag="ctx", name="cps")
                njt = 4 * i + 4
                LAG = 3  # scores/exp run this many j-tiles ahead of ctx

                pending = {}

                def emit_scores(jt):
                    mark(f"sc i{i} hp{hp} jt{jt}")
                    r = jt - 4 * i  # diagonal offset
                    lo = max(r, 0) * 128
                    scp = scps.tile([128, 2, 512], f32, tag="sc", name="scp")
                    for z, h in enumerate(heads):
                        po = 64 * z
                        nc.tensor.matmul(
                            scp[:, z, lo:512],
                            KTr_t[po:po + 64, to, ts(jt, 128)],
                            QT_t[po:po + 64, to, ds512(i, lo)],
                            start=True, stop=True)
                    prp = probsp.tile([128, 2, 512], bf16, tag="pr", name="prp")
                    nc.scalar.activation(prp[:, :, lo:512], scp[:, :, lo:512],
                                         AFT.Exp)
                    if r >= 0:
                        # Pool, not DVE: keeps the exp->mask->ctx chain clear
                        # of the DVE evac/norm queue.
                        nc.gpsimd.tensor_mul(
                            prp[:, :, lo:lo + 128], prp[:, :, lo:lo + 128],
                            mask_t[:])
                    pending[jt] = (lo, prp)

                def emit_ctx(jt):
                    mark(f"ctx i{i} hp{hp} jt{jt}")
                    lo, prp = pending.pop(jt)
                    for z, h in enumerate(heads):
                        # skip_group_check when normalizing piecewise: the
                        # per-bank group flags would otherwise forbid reading
                        # completed low columns while high columns still
                        # accumulate (disjoint addresses; deps stay correct).
                        nc.tensor.matmul(
                            cps[0:HD + 1, z, lo:512], V_t[:, jt, h, :],
                            prp[:, z, lo:512],
                            start=(jt == 0), stop=(jt == njt - 1),
                            skip_group_check=(on_piece is not None))

                def norm_piece(p):
                    # columns [128p, 128p+128) of this chunk are fully
                    # accumulated once ctx j-tile 4i+p has run (mid-group
                    # PSUM read; later j-tiles only touch higher columns).
                    # Per-z chains pipeline across DVE (recip, mul) and Pool
                    # (broadcast) for a shorter critical path.
                    mark(f"norm i{i} hp{hp} p{p}")
                    c0 = 128 * p
                    rec = probsp.tile([1, 2, 128], f32, tag="recp",
                                      name="recp", bufs=3)
                    bcs = probsp.tile([HD, 2, 128], f32, tag="bcsp",
                                      name="bcsp", bufs=3)
                    for z in range(2):
                        nc.vector.reciprocal(
                            rec[:, z, :], cps[HD:HD + 1, z, ds(c0, 128)])
                        nc.gpsimd.partition_broadcast(bcs[:, z, :],
                                                      rec[:, z, :])
                    for z in range(2):
                        po = 64 * z
                        nc.vector.tensor_mul(
                            ctxT_t[po:po + 64, to, ds(512 * i + c0, 128)],
                            cps[0:HD, z, ds(c0, 128)], bcs[:, z, :])

                def maybe_piece(jt_done):
                    if on_piece is not None and jt_done >= 4 * i:
                        p = jt_done - 4 * i
                        norm_piece(p)
                        on_piece(p)

                filler = list(early_filler)
                for jt in range(njt):
                    if jt >= 2 and filler:
                        # PE filler for the exp-gated pipe-fill phase.  The
                        # PE runs in order, so the filler must be emitted
                        # BEFORE the scores matmul that will block on the
                        # exp-paced psum rotation.
                        filler.pop(0)()
                    emit_scores(jt)
                    if mid_hooks and jt in mid_hooks:
                        mid_hooks[jt]()
                    if jt >= LAG:
                        emit_ctx(jt - LAG)
                        maybe_piece(jt - LAG)
                for jt in range(njt - LAG, njt):
                    emit_ctx(jt)
                    maybe_piece(jt)

                if on_piece is not None:
                    return None

                # normalize straight out of PSUM: reciprocal of the ones-row,
                # broadcast down 64 partitions, then per-head multiplies into
                # the bf16 ctx tile.  Optionally deferred so later-emitted DVE
                # work (kq evacs) gets queue priority over this chain.
                def emit_norm():
                    mark(f"norm i{i} hp{hp}")
                    rec = probsp.tile([1, 2, 512], f32, tag="rec", name="rec",
                                      bufs=2)
                    nc.vector.reciprocal(rec[:], cps[HD:HD + 1, :, :])
                    bcs = probsp.tile([HD, 2, 512], f32, tag="bcs", name="bcs",
                                      bufs=2)
                    nc.gpsimd.partition_broadcast(bcs[:], rec[:])
                    for z, h in enumerate(heads):
                        po = 64 * z
                        nc.vector.tensor_mul(
                            ctxT_t[po:po + 64, to, ts(i, 512)],
                            cps[0:HD, z, :], bcs[:, z, :])

                if defer_norm:
                    return emit_norm
                emit_norm()
                return None

            def proj_group(m, last=False, split_dma=False, act_ok=True,
                           rotate=False):
                # both 512-column halves of output row-tile m: two psum
                # accumulations, two Pool evac copies into one fp16 staging
                # tile, a single SP-issued DMA.  split_dma sends each half on
                # its own queue as soon as its copy lands — used for the very
                # last tile so the final transfer is half-size.
                ot = osbp.tile([128, 2, 512], f16, tag="ot", name="ot")
                # with split_dma, n=1 goes first: its DVE copy is the longer
                # pole of the final drain, so start it as early as possible
                for n in ((1, 0) if split_dma else (0, 1)):
                    mark(f"proj m{m} n{n}")
                    if last or rotate:
                        # ctx pool is excluded: the last pair's cps tile is
                        # still being read by the piecewise normalize.
                        pool, tg = ((mmps, "mm"), (scps, "sc"))[(2 * m + n) % 2]
                    else:
                        pool, tg = mmps, "mm"
                    mm = pool.tile([128, 512], f32, tag=tg, name="mm")
                    for t in range(2):
                        nc.tensor.matmul(
                            mm[:], ctxT_t[:, t, ts(m, 128)],
                            wo_t[:, t, ts(n, 512)],
                            start=(t == 0), stop=(t == 1))
                    # PSUM evacuation: only ACT and DVE can read PSUM (BIR
                    # verifier rejects GPSIMD).  Split across both so the two
                    # psum tiles of a group free in parallel; act_ok=False
                    # keeps ACT clear for exp in attention-saturated phases.
                    if n == 0 and act_ok:
                        nc.scalar.copy(ot[:, n, :], mm[:])
                    else:
                        nc.vector.tensor_copy(ot[:, n, :], mm[:])
                    if split_dma:
                        eng = nc.sync if n == 0 else nc.scalar
                        eng.dma_start(out_d[ts(m, 128), ts(n, 512)],
                                      ot[:, n, :])
                if not split_dma:
                    nc.sync.dma_start(out_d[ts(m, 128), :], ot[:])

            def prep(i):
                kq_group(wk_t, KTr_t, 0, i)
                kq_group(wq_t, QT_t, 0, i)
                for jt in range(4 * i, 4 * i + 4):
                    v_group(jt)

            prep(0)
            for i in range(SC):
                # kq t1 between the pairs fills pair (i,1)'s exp-gated warmup
                # with PE work; pair (i,0)'s norm chain is emitted after
                # kq t1 so the Q evac copy gets DVE queue priority.
                norm0 = attn_pair(i, 0, defer_norm=True)
                kq_group(wk_t, KTr_t, 1, i)
                kq_group(wq_t, QT_t, 1, i)
                if i == SC - 1:
                    def last_piece(p, i=i):
                        # chunks 0 and SC-2's deferred projections ride along
                        # as PE filler while each piece's norm chain resolves;
                        # they go first so their DMAs clear the queue before
                        # the piece's own output (which ends the kernel).
                        proj_group(4 * (i - 1) + p)
                        proj_group(p)
                        proj_group(4 * i + p, last=True,
                                   split_dma=(p == 3))
                    # chunk SC-3's deferred projection fills the exp-gated
                    # pipe-fill phase of this final pair; act_ok=False so
                    # these copies never stall the exp stream.
                    fillers = [lambda m=m: proj_group(m, act_ok=False)
                               for m in range(4 * (i - 2), 4 * (i - 2) + 4)]
                    norm0()
                    attn_pair(i, 1, on_piece=last_piece, early_filler=fillers)
                else:
                    norm0()
                    attn_pair(i, 1)
                    prep(i + 1)

    nc.compile()
    return nc


def _causal_mask():
    # boundary block: mask[p, z, c] = 1.0 where key p <= query c (within a
    # 128 block); stored per-head-pair (z dim) for the packed layout
    p = np.arange(128)[:, None]
    c = np.arange(128)[None, :]
    m = (p <= c).astype(ml_dtypes.bfloat16)
    return np.ascontiguousarray(np.broadcast_to(m[:, None, :], (128, 2, 128)))


def make_in_maps(x, Wq, Wk, Wv, Wo):
    """Per-core input dict list (core = b * NG + g)."""
    bft = ml_dtypes.bfloat16
    mask = _causal_mask()
    xT = [np.ascontiguousarray(x[b].T).astype(bft) for b in range(B)]
    in_maps = []
    for core in range(NCORES):
        b, g = divmod(core, NG)
        sl = slice(g * GO, (g + 1) * GO)
        in_maps.append({
            "xT": xT[b],
            "wqT": np.ascontiguousarray((Wq[sl, :] / np.sqrt(HD)).T).astype(bft),
            "wkT": np.ascontiguousarray(Wk[sl, :].T).astype(bft),
            "wvT": np.ascontiguousarray(Wv[sl, :].T).astype(bft),
            "woT": np.ascontiguousarray(Wo[:, sl].T).astype(bft),
            "mask": mask,
        })
    return in_maps


def kernel(x, Wq, Wk, Wv, Wo, bo):
    x = np.asarray(x, dtype=np.float32)
    Wq = np.asarray(Wq, dtype=np.float32)
    Wk = np.asarray(Wk, dtype=np.float32)
    Wv = np.asarray(Wv, dtype=np.float32)
    Wo = np.asarray(Wo, dtype=np.float32)
    bo = np.asarray(bo, dtype=np.float32)

    if "nc" not in _CACHE:
        _CACHE["nc"] = _build()
    nc = _CACHE["nc"]

    in_maps = make_in_maps(x, Wq, Wk, Wv, Wo)
    res = bass_utils.run_bass_kernel_spmd(nc, in_maps, core_ids=list(range(NCORES)))
    _CACHE["last_result"] = res

    out = np.zeros((B, S, D), dtype=np.float32)
    for core in range(NCORES):
        b = core // NG
        out[b] += res.results[core]["out"].astype(np.float32)
    out += bo
    return out
